# revision 33
# baseline (speedup 1.0000x reference)
"""Trainium2 Bass kernel for nn_CrossAttention (dense_transformer).

Sharding: data-parallel over batch B=8 across 8 NeuronCores (1 sample
per core). BatchNorm uses batch statistics, so per-channel partial
[sum, sumsq] are all-reduced across cores ([128,2] f32 payload, 2x).

Per-core layout: activations [C=128 partitions, N=H*W=2304 free]. All
heavy matmuls run in float32r (TF32-like, 1 PE cycle/row at free>=256,
4x fp32). Inputs/weights are DMA'd straight into float32r tiles (the
PE rounds on read); intermediate operands are produced by DVE/ACT ops
writing float32r. The attention value path (exp output, v^T, softmax
partial sums) runs in bf16: attention weights are positive/normalized
and gamma~0.1 scales their residual contribution, so the extra
rounding is negligible (measured end-to-end rel err 3.9e-4).

Attention in energy-transposed layout with algebraically folded
projections (saves the k-projection and all per-block transposes):
  energy^T[m,q] = sum_c yp[c,m] * qh[c,q],  qh = (Wq^T Wk)^T @ h
  v^T[mo]       = matmul(lhsT=yp[:,mo-chunk], rhs=gamma*Wv^T)
Softmax (no max-subtraction needed: |energy| <= ~10) reduces over m =
partitions: exp'd energies are pair-summed (two m-chunks share one
2-bank PSUM tile so each Exp activation covers both), tree-folded on
DVE/GPSIMD, and one ones-matmul per q-superblock does the partition
sum, arriving pre-broadcast across partitions; 1/colsum is applied
after the v-contraction (reciprocal_approx_fast). conv3x3 = 9
shifted-window matmuls over a zero-padded [128,50,50] buffer. Energy
matmuls are software-pipelined one pair ahead of exp/out-matmuls.

Measured on trn2 (neuron-profile via axon NTFF hook): ~263-280 us/core,
end-to-end rel err vs fp64 numpy reference: 3.9e-4.
"""

import sys

sys.path.insert(0, "/opt/trn_rl_repo")

import numpy as np

_NC_CACHE = {}

B, CIN, C, H, W = 8, 256, 128, 48, 48
N = H * W  # 2304
P = 128
NKO = CIN // P  # 2
NMO = N // P  # 18
# q superblocks: row-aligned chunks (48-col rows); 480 = 10 rows
QCH = [(0, 480), (480, 480), (960, 480), (1440, 480), (1920, 384)]
ROWCH = [(0, 10), (10, 10), (20, 10), (30, 10), (40, 8)]
NSTAT = float(B * N)  # BN stat count over (B,H,W)
EPS = 1e-5


def _build(variant="default"):
    """variant: 'default' = 8-core w/ collectives; 'sim' = single-core,
    collectives replaced by DMA copy (for TimelineSim profiling)."""
    key = f"nc_{variant}"
    if key in _NC_CACHE:
        return _NC_CACHE[key]

    import concourse.mybir as mybir
    import concourse.tile as tile
    from concourse import bacc
    from contextlib import ExitStack

    F32 = mybir.dt.float32
    F32R = mybir.dt.float32r
    BF16 = mybir.dt.bfloat16
    AF = mybir.ActivationFunctionType
    ALU = mybir.AluOpType
    AX = mybir.AxisListType

    sim = variant == "sim"
    import os as _os
    if _os.environ.get("KERNEL_LDW_OPT", "0") == "1":
        _enable_ldw_opt()
    nc = bacc.Bacc(
        "TRN2", target_bir_lowering=False, debug=False,
        num_devices=1 if sim else 8,
    )

    # ---- DRAM I/O (f32r tensors carry plain fp32 bits; PE rounds) ----
    d_x = nc.dram_tensor("x", [CIN, N], F32R, kind="ExternalInput")
    d_y = nc.dram_tensor("y", [CIN, N], F32R, kind="ExternalInput")
    d_w_inT = nc.dram_tensor("w_inT", [P, NKO, P], F32R, kind="ExternalInput")
    d_b_in = nc.dram_tensor("b_in", [P, 1], F32, kind="ExternalInput")
    d_A1 = nc.dram_tensor("A1", [P, P], F32R, kind="ExternalInput")
    d_gwv1T = nc.dram_tensor("gwv1T", [P, P], F32R, kind="ExternalInput")
    d_A2 = nc.dram_tensor("A2", [P, P], F32R, kind="ExternalInput")
    d_gwv2T = nc.dram_tensor("gwv2T", [P, P], F32R, kind="ExternalInput")
    d_w1T = nc.dram_tensor("w1T", [P, 9, P], F32R, kind="ExternalInput")
    d_bn1s = nc.dram_tensor("bn1s", [P, 1], F32, kind="ExternalInput")
    d_bn1b = nc.dram_tensor("bn1b", [P, 1], F32, kind="ExternalInput")
    d_w2T = nc.dram_tensor("w2T", [P, 9, P], F32R, kind="ExternalInput")
    d_bn2s = nc.dram_tensor("bn2s", [P, 1], F32, kind="ExternalInput")
    d_bn2b = nc.dram_tensor("bn2b", [P, 1], F32, kind="ExternalInput")
    d_predT = nc.dram_tensor("predT", [P, P], F32R, kind="ExternalInput")
    d_pred_b = nc.dram_tensor("pred_b", [1, 1], F32, kind="ExternalInput")
    d_out = nc.dram_tensor("out", [1, N], F32, kind="ExternalOutput")

    with tile.TileContext(nc) as tc, ExitStack() as ctx:
        wgt = ctx.enter_context(tc.tile_pool(name="wgt", bufs=1))
        act = ctx.enter_context(tc.tile_pool(name="act", bufs=1))
        ew = ctx.enter_context(tc.tile_pool(name="ew", bufs=1))
        eeP = ctx.enter_context(tc.tile_pool(name="eeP", bufs=6))
        load = ctx.enter_context(tc.tile_pool(name="load", bufs=4))
        dram = ctx.enter_context(tc.tile_pool(name="dram", bufs=1, space="DRAM"))
        pE = ctx.enter_context(tc.tile_pool(name="pE", bufs=2, space="PSUM"))
        pO = ctx.enter_context(tc.tile_pool(name="pO", bufs=2, space="PSUM"))
        pS = ctx.enter_context(tc.tile_pool(name="pS", bufs=1, space="PSUM"))
        pM = ctx.enter_context(tc.tile_pool(name="pM", bufs=1, space="PSUM"))

        # ---------- weights (direct DMA; PE rounds f32r on read) ----------
        def load_w(dsrc, shape, tag, dtype=F32R):
            t = wgt.tile(shape, dtype, tag=tag)
            nc.sync.dma_start(t[:], dsrc[...])
            return t

        w_inT_r = load_w(d_w_inT, [P, NKO, P], "w_inT_r")
        A1_r = load_w(d_A1, [P, P], "A1_r")
        gwv1T_r = load_w(d_gwv1T, [P, P], "gwv1T_r")
        A2_r = load_w(d_A2, [P, P], "A2_r")
        gwv2T_r = load_w(d_gwv2T, [P, P], "gwv2T_r")
        w1T_r = load_w(d_w1T, [P, 9, P], "w1T_r")
        w2T_r = load_w(d_w2T, [P, 9, P], "w2T_r")
        predT_r = load_w(d_predT, [P, P], "predT_r")
        b_in = load_w(d_b_in, [P, 1], "b_in", F32)
        bn1s = load_w(d_bn1s, [P, 1], "bn1s", F32)
        bn1b = load_w(d_bn1b, [P, 1], "bn1b", F32)
        bn2s = load_w(d_bn2s, [P, 1], "bn2s", F32)
        bn2b = load_w(d_bn2b, [P, 1], "bn2b", F32)
        pred_b = load_w(d_pred_b, [1, 1], "pred_b", F32)

        ones_f = wgt.tile([P, P], F32, tag="ones_f")
        nc.gpsimd.memset(ones_f[:], 1.0)
        ones_b = wgt.tile([P, P], BF16, tag="ones_b")
        nc.vector.tensor_copy(ones_b[:], ones_f[:])

        zrow = wgt.tile([P, W + 2], F32, tag="zrow")
        nc.gpsimd.memset(zrow[:], 0.0)

        # ---------- stage A: load x,y per q-chunk (interleaved) so the
        # first projections/energies start ~5us in; DMA tail overlaps attn1
        xr = [load.tile([P, N], F32R, tag="in_r", name=f"xr{k}") for k in range(NKO)]
        yr = [load.tile([P, N], F32R, tag="in_r", name=f"yr{k}") for k in range(NKO)]
        xp = act.tile([P, N], F32R, tag="tagA")
        yp = act.tile([P, N], F32R, tag="tagB")
        for q0, qn in QCH:
            for dsrc, rr in ((d_x, xr), (d_y, yr)):
                for ko in range(NKO):
                    nc.sync.dma_start(
                        rr[ko][:, q0 : q0 + qn],
                        dsrc[ko * P : (ko + 1) * P, q0 : q0 + qn],
                    )
            for rr, dst in ((xr, xp), (yr, yp)):
                ps = pM.tile([P, 480], F32, tag="mps")
                for ko in range(NKO):
                    nc.tensor.matmul(
                        ps[:, :qn],
                        w_inT_r[:, ko, :],
                        rr[ko][:, q0 : q0 + qn],
                        start=(ko == 0),
                        stop=(ko == NKO - 1),
                    )
                nc.vector.tensor_scalar_add(dst[:, q0 : q0 + qn], ps[:, :qn], b_in[:])

        # ---------- helpers ----------
        def project(lhs_r, src_r, dst_tag, dtype=F32R):
            dst = act.tile([P, N], dtype, tag=dst_tag)
            for q0, qn in QCH:
                ps = pM.tile([P, 480], F32, tag="mps")
                nc.tensor.matmul(
                    ps[:, :qn], lhs_r[:], src_r[:, q0 : q0 + qn], start=True, stop=True
                )
                nc.vector.tensor_copy(dst[:, q0 : q0 + qn], ps[:, :qn])
            return dst

        def build_vT(gwvT_r, vT_tag):
            # vT[mo][m, c] = sum_c' yp[c', mo*P+m] * (gamma*wv^T)[c', c]
            # = one matmul per m-chunk with yp as stationary: no transposes
            vT = act.tile([P, NMO, P], BF16, tag=vT_tag)
            for mo in range(NMO):
                pst = pM.tile([P, 480], F32, tag="mps")
                nc.tensor.matmul(
                    pst[:, :P], yp[:, mo * P : (mo + 1) * P], gwvT_r[:],
                    start=True, stop=True,
                )
                nc.vector.tensor_copy(vT[:, mo, :], pst[:, :P])
            return vT

        def zero_pad_border(pad):
            nc.vector.tensor_copy(pad[:, 0, :], zrow[:])
            nc.vector.tensor_copy(pad[:, H + 1, :], zrow[:])
            nc.vector.tensor_copy(pad[:, 1 : H + 1, 0:1], zrow[:, :H, None])
            nc.vector.tensor_copy(pad[:, 1 : H + 1, W + 1 : W + 2], zrow[:, :H, None])

        def attention(qh_r, vT_r, resid_r, pad_tag):
            pad = act.tile([P, H + 2, W + 2], F32R, tag=pad_tag)
            zero_pad_border(pad)
            NPAIR = NMO // 2  # 9
            steps = [(qi, j) for qi in range(len(QCH)) for j in range(NPAIR)]

            def emit_energy(qi, j):
                q0, qn = QCH[qi]
                mo0, mo1 = 2 * j, 2 * j + 1
                ps_e = pE.tile([P, 1024], F32, tag="energy")
                nc.tensor.matmul(
                    ps_e[:, :qn],
                    yp[:, mo0 * P : (mo0 + 1) * P],
                    qh_r[:, q0 : q0 + qn],
                    start=True, stop=True,
                )
                nc.tensor.matmul(
                    ps_e[:, 512 : 512 + qn],
                    yp[:, mo1 * P : (mo1 + 1) * P],
                    qh_r[:, q0 : q0 + qn],
                    start=True, stop=True,
                )
                return ps_e

            # software pipeline: energies one pair ahead of exp/out
            pend = emit_energy(*steps[0])
            ps_o = ps_s = None
            for idx, (qi, j) in enumerate(steps):
                q0, qn = QCH[qi]
                mo0, mo1 = 2 * j, 2 * j + 1
                ps_e = pend
                pend = emit_energy(*steps[idx + 1]) if idx + 1 < len(steps) else None
                if j == 0:
                    ps_o = pO.tile([P, 480], F32, tag="attn_out")
                    ps_s = pS.tile([P, 480], F32, tag="colsum")
                    prs = []
                # one Exp covers both chunks (gap cols never read)
                ee = eeP.tile([P, 1024], BF16, tag="ee")
                nc.scalar.activation(ee[:, : 512 + qn], ps_e[:, : 512 + qn], AF.Exp)
                nc.tensor.matmul(
                    ps_o[:, :qn], vT_r[:, mo0, :], ee[:, :qn],
                    start=(j == 0), stop=False,
                )
                nc.tensor.matmul(
                    ps_o[:, :qn], vT_r[:, mo1, :], ee[:, 512 : 512 + qn],
                    start=False, stop=(j == NPAIR - 1),
                )
                # pair-sum, folded incrementally (binary counter; <=4 live
                # tiles) on DVE/GPSIMD; one partition-sum matmul per qsb
                pr = eeP.tile([P, 480], BF16, tag="pair")
                eng = nc.gpsimd if j % 3 == 1 else nc.vector
                eng.tensor_tensor(
                    pr[:, :qn], ee[:, :qn], ee[:, 512 : 512 + qn], ALU.add
                )
                lv, t = 0, pr
                while prs and prs[-1][0] == lv:
                    prev = prs.pop()[1]
                    o = eeP.tile([P, 480], BF16, tag="fold")
                    e2 = nc.gpsimd if (j + lv) % 3 == 2 else nc.vector
                    e2.tensor_tensor(o[:, :qn], prev[:, :qn], t[:, :qn], ALU.add)
                    t, lv = o, lv + 1
                prs.append((lv, t))
                if j == NPAIR - 1:
                    while len(prs) > 1:
                        (_, a), (_, b2) = prs.pop(), prs.pop()
                        o = eeP.tile([P, 480], BF16, tag="fold")
                        nc.vector.tensor_tensor(
                            o[:, :qn], a[:, :qn], b2[:, :qn], ALU.add
                        )
                        prs.append((99, o))
                    nc.tensor.matmul(
                        ps_s[:, :qn], ones_b[:], prs.pop()[1][:, :qn],
                        start=True, stop=True,
                    )
                    rcp = ew.tile([P, 480], F32, tag="recip")
                    nc.vector.reciprocal_approx_fast(rcp[:, :qn], ps_s[:, :qn])
                    tmp = ew.tile([P, 480], F32, tag="tmp")
                    nc.vector.tensor_tensor(
                        tmp[:, :qn], ps_o[:, :qn], rcp[:, :qn], ALU.mult
                    )
                    r0, nr = q0 // W, qn // W
                    nc.vector.tensor_tensor(
                        pad[:, 1 + r0 : 1 + r0 + nr, 1 : W + 1],
                        tmp[:, :qn].rearrange("p (a b) -> p a b", b=W),
                        resid_r[:, q0 : q0 + qn].rearrange("p (a b) -> p a b", b=W),
                        ALU.add,
                    )
            return pad

        def conv_bn_relu(pad, wT_r, bns, bnb, t_tag, out_tag, ar_idx, overlap_fn=None):
            # conv3x3 SAME via 9 shifted-window matmuls; batch-stat allreduce
            t_sb = act.tile([P, N], F32, tag=t_tag)
            sums = ew.tile([P, len(ROWCH)], F32, tag="sums")
            sqs = ew.tile([P, len(ROWCH)], F32, tag="sqs")
            for ci, (r0, nr) in enumerate(ROWCH):
                qn = nr * W
                ps = pM.tile([P, 480], F32, tag="mps")
                t = 0
                for dy in range(3):
                    for dx in range(3):
                        nc.tensor.matmul(
                            ps[:, :qn],
                            wT_r[:, t, :],
                            pad[:, dy + r0 : dy + r0 + nr, dx : dx + W],
                            start=(t == 0),
                            stop=(t == 8),
                        )
                        t += 1
                q0 = r0 * W
                nc.vector.tensor_copy(t_sb[:, q0 : q0 + qn], ps[:, :qn])
                nc.vector.reduce_sum(sums[:, ci : ci + 1], ps[:, :qn], axis=AX.X)
                scr = ew.tile([P, 480], F32, tag="sq_scr")
                nc.scalar.activation(
                    scr[:, :qn], ps[:, :qn], AF.Square,
                    accum_out=sqs[:, ci : ci + 1],
                )
            stats = ew.tile([P, 2], F32, tag="stats")
            nc.vector.reduce_sum(stats[:, 0:1], sums[:], axis=AX.X)
            nc.vector.reduce_sum(stats[:, 1:2], sqs[:], axis=AX.X)
            nc.vector.tensor_scalar_mul(stats[:], stats[:], 1.0 / NSTAT)
            cc_in = dram.tile([P, 2], F32, tag=f"cc_in{ar_idx}")
            cc_out = dram.tile([P, 2], F32, tag=f"cc_out{ar_idx}")
            nc.sync.dma_start(cc_in[:], stats[:])
            if sim:
                nc.sync.dma_start(cc_out[:], cc_in[:])
            else:
                nc.gpsimd.collective_compute(
                    "AllReduce",
                    ALU.add,
                    replica_groups=[list(range(8))],
                    ins=[cc_in[:].opt()],
                    outs=[cc_out[:].opt()],
                )
            if overlap_fn is not None:
                overlap_fn()
            st_all = ew.tile([P, 2], F32, tag="st_all")
            nc.sync.dma_start(st_all[:], cc_out[:])
            mean = st_all[:, 0:1]
            var = ew.tile([P, 1], F32, tag="var")
            nc.vector.tensor_tensor(var[:], mean, mean, ALU.mult)
            # var = m2 - mean^2 + eps, fused: (var * -1 + m2) then +eps
            nc.vector.scalar_tensor_tensor(
                var[:], var[:], -1.0, st_all[:, 1:2], ALU.mult, ALU.add
            )
            nc.vector.tensor_scalar_add(var[:], var[:], EPS)
            std = ew.tile([P, 1], F32, tag="std")
            nc.scalar.activation(std[:], var[:], AF.Sqrt)
            a_sc = ew.tile([P, 1], F32, tag="a_sc")
            with nc.allow_low_precision(reason="bn rsqrt"):
                nc.vector.reciprocal(a_sc[:], std[:])
            nc.vector.tensor_tensor(a_sc[:], a_sc[:], bns[:], ALU.mult)
            c_bi = ew.tile([P, 1], F32, tag="c_bi")
            # c = bnb - mean*a, fused: (mean * -a??) -> (mean mult a) then rsub
            nc.vector.tensor_tensor(c_bi[:], mean, a_sc[:], ALU.mult)
            nc.vector.tensor_tensor(c_bi[:], bnb[:], c_bi[:], ALU.subtract)
            h_out = act.tile([P, N], F32R, tag=out_tag)
            for q0, qn in QCH:
                nc.scalar.activation(
                    h_out[:, q0 : q0 + qn], t_sb[:, q0 : q0 + qn],
                    AF.Relu, bias=c_bi[:], scale=a_sc[:],
                )
            return h_out

        # ---------- pipeline ----------
        # tag sharing (bufs=1 slots reused via WAR deps): vfT: v1f->v2f;
        # tagC: qh1->qh2; tagT: t1->t2; tagE: h1pad->h3pad; h2: h2->r2
        vT1 = build_vT(gwv1T_r, "vT1")
        qh1 = project(A1_r, xp, "tagC")
        h1pad = attention(qh1, vT1, xp, "tagE")
        vT2_box = []
        h2 = conv_bn_relu(
            h1pad, w1T_r, bn1s, bn1b, "tagT", "h2", 1,
            overlap_fn=lambda: vT2_box.append(build_vT(gwv2T_r, "vT2")),
        )
        vT2 = vT2_box[0]
        qh2 = project(A2_r, h2, "tagC")
        h3pad = attention(qh2, vT2, h2, "tagE")
        r2 = conv_bn_relu(h3pad, w2T_r, bn2s, bn2b, "tagT", "h2", 2)

        # ---------- pred head ----------
        out_sb = act.tile([1, N], F32, tag="out_sb")
        for q0, qn in QCH:
            ps = pM.tile([P, 480], F32, tag="mps")
            nc.tensor.matmul(
                ps[:, :qn], predT_r[:], r2[:, q0 : q0 + qn], start=True, stop=True
            )
            nc.vector.tensor_scalar_add(
                out_sb[:, q0 : q0 + qn], ps[0:1, :qn], pred_b[:]
            )
        nc.sync.dma_start(d_out[:, :], out_sb[:])

    nc.compile()
    _NC_CACHE[key] = nc
    return nc


def _enable_ldw_opt():
    """Rewrite walrus --enable-ldw-opt flag (elide/overlap redundant PE
    weight loads). bass_utils hardcodes false; patch its run_command."""
    import concourse.bass_utils as _bu

    if getattr(_bu, "_ldw_patched", False):
        return
    _orig = _bu.run_command

    def _patched(argv, **kw):
        argv = [
            a.replace("--enable-ldw-opt=false", "--enable-ldw-opt=true")
            if isinstance(a, str) else a
            for a in argv
        ]
        return _orig(argv, **kw)

    _bu.run_command = _patched
    _bu._ldw_patched = True


def _install_ntff_hook():
    """Register the axon NTFF profiling hook (antenv.axon_hooks is absent
    in this image; libaxon_pjrt.so exports the C ABI — same wiring as
    trn_agent_boot's _ntff_profile_via_ctypes)."""
    import sys as _sys, types, ctypes, contextlib

    if "antenv.axon_hooks" in _sys.modules:
        return
    try:
        lib = ctypes.CDLL("/opt/axon/libaxon_pjrt.so")
        lib.axon_start_nrt_profile.argtypes = [
            ctypes.POINTER(ctypes.c_int64), ctypes.c_size_t,
        ]
        lib.axon_start_nrt_profile.restype = ctypes.c_int64
        lib.axon_stop_nrt_profile.argtypes = [ctypes.c_char_p]
        lib.axon_stop_nrt_profile.restype = ctypes.c_int64
    except (OSError, AttributeError):
        return

    @contextlib.contextmanager
    def _hook(output_dir, device_ids):
        import jax

        jax.devices()
        if device_ids:
            ids = (ctypes.c_int64 * len(device_ids))(*device_ids)
            rc = lib.axon_start_nrt_profile(ids, len(device_ids))
        else:
            rc = lib.axon_start_nrt_profile(None, 0)
        if rc != 0:
            raise RuntimeError(f"axon_start_nrt_profile rc={rc}")
        try:
            yield
        finally:
            n = lib.axon_stop_nrt_profile(str(output_dir).encode())
            if n < 0:
                raise RuntimeError(f"axon_stop_nrt_profile rc={n}")

    mod = types.ModuleType("antenv.axon_hooks")
    mod.get_axon_ntff_profile_hook = lambda: _hook
    mod.set_axon_ntff_profile_hook = lambda h: None
    _sys.modules["antenv.axon_hooks"] = mod
    # artifact upload has no bucket in this container; keep files local
    import concourse.bass_utils as _bu

    _bu.upload_artifacts = lambda d: d


def kernel(**inputs):
    from concourse.bass_utils import run_bass_kernel_spmd
    import os

    nc = _build()

    f32 = np.float32
    x = np.ascontiguousarray(inputs["x"], dtype=f32).reshape(B, CIN, N)
    y = np.ascontiguousarray(inputs["y"], dtype=f32).reshape(B, CIN, N)
    w_in = np.asarray(inputs["w_in"], dtype=f32)
    b_in = np.asarray(inputs["b_in"], dtype=f32).reshape(P, 1)
    ca_wq = np.asarray(inputs["ca_wq"], dtype=f32)
    ca_wk = np.asarray(inputs["ca_wk"], dtype=f32)
    ca_wv = np.asarray(inputs["ca_wv"], dtype=f32)
    g1 = np.full((P, 1), np.asarray(inputs["ca_gamma"], dtype=f32).reshape(-1)[0], f32)
    sa_wq = np.asarray(inputs["sa_wq"], dtype=f32)
    sa_wk = np.asarray(inputs["sa_wk"], dtype=f32)
    sa_wv = np.asarray(inputs["sa_wv"], dtype=f32)
    g2 = np.full((P, 1), np.asarray(inputs["sa_gamma"], dtype=f32).reshape(-1)[0], f32)
    conv1_w = np.asarray(inputs["conv1_w"], dtype=f32)
    conv2_w = np.asarray(inputs["conv2_w"], dtype=f32)
    bn1s = np.asarray(inputs["bn1_s"], dtype=f32).reshape(P, 1)
    bn1b = np.asarray(inputs["bn1_b"], dtype=f32).reshape(P, 1)
    bn2s = np.asarray(inputs["bn2_s"], dtype=f32).reshape(P, 1)
    bn2b = np.asarray(inputs["bn2_b"], dtype=f32).reshape(P, 1)
    pred_w = np.asarray(inputs["pred_w"], dtype=f32)
    pred_b = np.asarray(inputs["pred_b"], dtype=f32).reshape(1, 1)

    # host-side weight prep (small, O(C^2))
    w_inT = np.ascontiguousarray(
        w_in.T.reshape(NKO, P, P).transpose(1, 0, 2)
    )  # [cin_p, ko, cout]
    A1 = np.ascontiguousarray(ca_wq.T @ ca_wk)
    A2 = np.ascontiguousarray(sa_wq.T @ sa_wk)
    wv1T = np.ascontiguousarray(ca_wv.T)
    wv2T = np.ascontiguousarray(sa_wv.T)
    # conv taps: [o, i, 3, 3] -> lhsT per tap [i, o]; layout [i_p, tap, o]
    w1T = np.ascontiguousarray(
        conv1_w.transpose(2, 3, 1, 0).reshape(9, P, P).transpose(1, 0, 2)
    )
    w2T = np.ascontiguousarray(
        conv2_w.transpose(2, 3, 1, 0).reshape(9, P, P).transpose(1, 0, 2)
    )
    predT = np.zeros((P, P), f32)
    predT[:, 0] = pred_w[0]

    shared = {
        "w_inT": w_inT, "b_in": b_in, "A1": A1,
        "gwv1T": np.ascontiguousarray(g1[0, 0] * wv1T),
        "A2": A2, "gwv2T": np.ascontiguousarray(g2[0, 0] * wv2T),
        "w1T": w1T, "bn1s": bn1s,
        "bn1b": bn1b, "w2T": w2T, "bn2s": bn2s, "bn2b": bn2b,
        "predT": predT, "pred_b": pred_b,
    }
    in_maps = [
        {"x": np.ascontiguousarray(x[i]), "y": np.ascontiguousarray(y[i]), **shared}
        for i in range(B)
    ]

    trace = bool(int(os.environ.get("KERNEL_TRACE", "0")))
    if trace:
        _install_ntff_hook()
    res = run_bass_kernel_spmd(nc, in_maps, core_ids=list(range(B)), trace=trace)
    if trace:
        _NC_CACHE["last_results"] = res
    out = np.stack(
        [res.results[i]["out"].reshape(1, H, W) for i in range(B)]
    ).astype(f32)
    return out
